# revision 46
# baseline (speedup 1.0000x reference)
"""AtnConv (contextual attention) kernel for 8 TRN2 NeuronCores.

Math (per image):
  P2 = 3x3 patches of x2, [L=4096, 1152]; Wn = P2 / max(||P2||, 1e-4)
  pooled*10 = (boxsum3x3(P2) * 90/cnt) @ Wn^T     (pooling is linear in queries)
  att = softmax_l(pooled*10)
  y = fold3x3(att @ P1)  ==  attD @ X1   where attD is the 3x3 *diagonal*
      sum  attD[q, m] = sum_{d in 3x3} att[q+d, m+d]  (spatial shifts of both
      the query and key grids).  This cuts the value GEMM contraction from
      1152 (patch dim) to 128 (channels) x 3 dx-shifted terms.

Sharding: core c -> image c//4, query-row block [16*(c%4), 16*(c%4)+16).
No collectives; the 1-query-row halo the diagonal sum needs across cores is
handled by emitting overlapped partial outputs (1152 = 1024 + 2*64 halo rows)
that the host accumulates.

Device pipeline per core:
  phase1: S_chunk = qbox @ WnT_chunk (float32r matmuls, full fp32 rate),
          chunk-softmax: e = exp(S - chunkmax) -> bf16 spill to DRAM,
          chunkmax/chunksum kept in SBUF.
  phase2: per m-tile: reload spill into a 66-grid padded layout, normalize
          in-place (exp-correction x 1/sum via ACT), dy-diagonal sum via
          partition-offset adds on DVE+Pool, PE-transpose to attD^T-partial,
          then y^T[c, q] = sum_dx X1dx^T @ A^T[:, q+dx]  (bf16 matmuls,
          contraction over the 4480-padded key grid in 35 k-tiles).
"""
import numpy as np
import ml_dtypes
from contextlib import ExitStack

import concourse.bass as bass
import concourse.bacc as bacc
import concourse.tile as tile
import concourse.mybir as mybir
from concourse.bass_utils import run_bass_kernel_spmd

B, H, W, C = 2, 64, 64, 128
K = 3
KKC = K * K * C          # 1152
L = H * W                # 4096
NCORES = 8
SH = 4                   # row-blocks per image
RS = H // SH             # 16 rows per block
MQ = RS * W              # 1024 queries per core
EPS = 1e-4
SCALE = 10.0

_F32 = mybir.dt.float32
_F32R = mybir.dt.float32r
_BF16 = mybir.dt.bfloat16

_cache = {}

NL = 512                 # l-chunk for GEMM1 / spill
NCH = L // NL            # 8 chunks
NK = KKC // 128          # 9 contraction tiles
NM = MQ // 128           # 8 query tiles
NA = NM + 1              # 9 qa-tiles (1024 + 2*64 halo)
QA = NA * 128            # 1152 A rows
PG = 66                  # padded grid pitch
AW = PG * PG             # 4356 = A real width
AWP = 4480               # A padded width (35 k-tiles of 128)
NLT = AWP // 128         # 35 contraction tiles for GEMM2
PQW = 18 * PG            # 1188 pitched output cols (18 query rows x 66)
ATF = PQW + 68           # 1256 A^T tile flat width (lead zero + row-18 pad)
NQC = 3                  # GEMM2 q-chunks
QC = PQW // NQC          # 396 cols per chunk
DY0 = 64                 # first live A col (X1 rows below are zero)
DY1 = 4292               # end of live A cols


def _build(bufcfg=None):
    bufcfg = bufcfg or {}
    nc = bacc.Bacc("TRN2", target_bir_lowering=False, debug=False,
                   enable_asserts=False, num_devices=NCORES)
    qboxT = nc.dram_tensor("qboxT", [KKC, MQ], _F32R, kind="ExternalInput").ap()
    wnT = nc.dram_tensor("wnT", [KKC, L], _F32R, kind="ExternalInput").ap()
    # x1 shifted copies, host-prepared as [35, 128, 128] (lt, l'-row, c)
    x1m = nc.dram_tensor("x1m", [NLT, 128, C], _BF16, kind="ExternalInput").ap()
    x1z = nc.dram_tensor("x1z", [NLT, 128, C], _BF16, kind="ExternalInput").ap()
    x1p = nc.dram_tensor("x1p", [NLT, 128, C], _BF16, kind="ExternalInput").ap()
    ident = nc.dram_tensor("ident", [128, 128], _BF16,
                           kind="ExternalInput").ap()
    yout = nc.dram_tensor("y", [C, PQW], _F32, kind="ExternalOutput").ap()
    espill = nc.dram_tensor("espill", [MQ, L], _BF16).ap()

    with tile.TileContext(nc, trace_sim=False) as tc:
        with (
            tc.tile_pool(name="cpool", bufs=1) as cpool,
            tc.tile_pool(name="statL", bufs=1) as statL,
            tc.tile_pool(name="srowp", bufs=1) as srowp,
            tc.tile_pool(name="stat2", bufs=8) as stat2,
        ):
            idt = cpool.tile([128, 128], _BF16, tag="ident")
            x1bigs = [cpool.tile([128, NLT * C], _BF16, tag=f"x1_{di}",
                                 name=f"x1b{di}") for di in range(3)]
            x1tiles = [[b[:, lt * C:(lt + 1) * C] for lt in range(NLT)]
                       for b in x1bigs]

            def load_x1(di):
                # staggered across early phase-2 m-iterations so the espill
                # reload transfers keep winning the DMA queue
                x1d = (x1m, x1z, x1p)[di]
                if di == 0:
                    nc.sync.dma_start(idt[:], ident[:])
                nc.sync.dma_start(
                    x1bigs[di][:, :].rearrange("p (k c) -> p k c", k=NLT),
                    x1d[:, :, :].rearrange("k p c -> p k c"))

            cmxs = [statL.tile([128, NCH], _F32, tag=f"cmx{m}", name=f"cmx{m}")
                    for m in range(NM)]
            csums = [statL.tile([128, NCH], _F32, tag=f"csum{m}", name=f"csum{m}")
                     for m in range(NM)]
            srows = [srowp.tile([128, L], _BF16, tag=f"srow{i}",
                                name=f"srow{i}") for i in range(5)]
            scales = []

            def make_scale(m):
                gmx = stat2.tile([128, 1], _F32, tag="gmx")
                nc.vector.reduce_max(gmx[:], cmxs[m][:, :],
                                     axis=mybir.AxisListType.X)
                ngmx = stat2.tile([128, 1], _F32, tag="ngmx")
                nc.scalar.mul(ngmx[:], gmx[:], -1.0)
                corr = stat2.tile([128, NCH], _F32, tag="corr")
                nc.scalar.activation(corr[:], cmxs[m][:, :],
                                     mybir.ActivationFunctionType.Exp,
                                     bias=ngmx[:], scale=1.0)
                wsum = stat2.tile([128, NCH], _F32, tag="wsum")
                nc.vector.tensor_mul(wsum[:], csums[m][:, :], corr[:])
                sm = stat2.tile([128, 1], _F32, tag="sm")
                nc.vector.reduce_sum(sm[:], wsum[:],
                                     axis=mybir.AxisListType.X)
                r = stat2.tile([128, 1], _F32, tag="r")
                nc.vector.reciprocal(r[:], sm[:])
                sc = statL.tile([128, NCH], _F32, tag=f"scale{m}",
                                name=f"scale{m}")
                nc.vector.tensor_scalar_mul(sc[:], corr[:], r[:])
                scales.append(sc)

            atts = []
            for i in range(3):
                t = srowp.tile([128, 68, PG], _BF16, tag=f"att{i}",
                               name=f"att{i}")
                nc.gpsimd.memset(t[:, 0:2, :], 0.0)
                nc.gpsimd.memset(t[:, 66:68, :], 0.0)
                nc.gpsimd.memset(t[:, 2:66, 0:1], 0.0)
                nc.gpsimd.memset(t[:, 2:66, 65:66], 0.0)
                atts.append(t)

            def rescale_m(m):
                sr = srows[m % 5]
                at = atts[m % 3]
                for n2 in range(NCH):
                    seg = at[:, 2 + 8 * n2:10 + 8 * n2, 1:65]
                    nc.vector.tensor_scalar_mul(
                        seg, sr[:, NL * n2:NL * (n2 + 1)],
                        scales[m][:, n2:n2 + 1])

            # ---------- phase 1: chunked GEMM1 + chunk softmax -> spill ----
            with (
                tc.tile_pool(name="qpool", bufs=1) as qpool,
                tc.tile_pool(name="wpool", bufs=bufcfg.get("wpool", 2)) as wpool,
                tc.tile_pool(name="psum1", bufs=bufcfg.get("psum1", 6), space="PSUM") as psum1,
                tc.tile_pool(name="estage", bufs=bufcfg.get("estage", 4)) as epool1,
                tc.tile_pool(name="stat1", bufs=8) as stat1,
            ):
                qbig = qpool.tile([128, NK * MQ], _F32R, tag="qbig",
                                  name="qbig")
                wchs = [wpool.tile([128, NK * NL], _F32R, tag=f"wch{i}",
                                   name=f"wch{i}") for i in range(2)]

                def load_wch(n):
                    # one DMA per chunk: [9 k-tiles x 128 rows x 512 cols]
                    nc.sync.dma_start(
                        wchs[n % 2][:, :].rearrange("p (k c) -> p k c", k=NK),
                        wnT[:, n * NL:(n + 1) * NL].rearrange(
                            "(k p) c -> p k c", k=NK))

                # qboxT m-major, one 3D-AP DMA per m-block; the m=0 block
                # and a per-k split of wch chunk 0 go first so the first
                # matmul can issue at ~3us and the PE p-state ramp starts
                # sooner
                qdst = qbig[:, :].rearrange("p (k q) -> p k q", k=NK)

                def load_qbox(m):
                    nc.sync.dma_start(
                        qdst[:, :, m * 128:(m + 1) * 128],
                        qboxT[:, m * 128:(m + 1) * 128].rearrange(
                            "(k p) c -> p k c", k=NK))

                load_qbox(0)
                for k in range(NK):
                    nc.sync.dma_start(
                        wchs[0][:, k * NL:(k + 1) * NL],
                        wnT[k * 128:(k + 1) * 128, 0:NL])
                for m in range(1, NM):
                    load_qbox(m)
                for n in range(NCH):
                    if n + 1 < NCH:
                        load_wch(n + 1)
                    wch = wchs[n % 2]
                    for m in range(NM):
                        ps = psum1.tile([128, NL], _F32)
                        for k in range(NK):
                            nc.tensor.matmul(
                                ps[:],
                                qbig[:, k * MQ + m * 128:
                                     k * MQ + (m + 1) * 128],
                                wch[:, k * NL:(k + 1) * NL],
                                start=(k == 0), stop=(k == NK - 1))
                        nc.vector.reduce_max(cmxs[m][:, n:n + 1], ps[:],
                                             axis=mybir.AxisListType.X)
                        ncmx = stat1.tile([128, 1], _F32, tag="ncmx")
                        nc.scalar.mul(ncmx[:], cmxs[m][:, n:n + 1], -1.0)
                        est = epool1.tile([128, NL], _BF16)
                        nc.scalar.activation(
                            est[:], ps[:], mybir.ActivationFunctionType.Exp,
                            bias=ncmx[:], scale=1.0,
                            accum_out=csums[m][:, n:n + 1])
                        nc.sync.dma_start(
                            espill[m * 128:(m + 1) * 128,
                                   n * NL:(n + 1) * NL], est[:])
                        if n == NCH - 1:
                            # phase-2 warm-up: pre-dispatch the espill
                            # reload + softmax scale for the first m-tiles
                            if m <= 4:
                                nc.scalar.dma_start(
                                    srows[m % 5][:],
                                    espill[m * 128:(m + 1) * 128, :])
                            if m <= 2:
                                make_scale(m)
                            if 3 <= m <= 5:
                                rescale_m(m - 3)
            # ---------- phase 2: normalize + dy-diagsum + transpose + GEMM2 -
            with (
                tc.tile_pool(name="apool", bufs=1) as apool,
                tc.tile_pool(name="atp", bufs=1) as atp,
                tc.tile_pool(name="psumT", bufs=bufcfg.get("psumT", 6), space="PSUM") as psumT,
                tc.tile_pool(name="psum2", bufs=bufcfg.get("psum2", 2), space="PSUM") as psum2,
                tc.tile_pool(name="ypool", bufs=1) as ypool,
            ):
                # att tiles in 68x66 padded grid; pads zeroed once per
                # buffer (on the idle Pool engine).
                # A (dy-summed) tiles, flat [128, 4480]; tail zeroed once.
                ats = []
                for i in range(4):
                    t = apool.tile([128, AWP], _BF16, tag=f"abuf{i}",
                                   name=f"abuf{i}")
                    # dead A cols (X1 rows there are zero): zero once so
                    # the transposes never feed NaN garbage into the GEMM
                    nc.gpsimd.memset(t[:, 0:DY0], 0.0)
                    nc.gpsimd.memset(t[:, DY1:AWP], 0.0)
                    ats.append(t)
                tmps = []
                for i in range(2):
                    t = apool.tile([128, AW], _BF16, tag=f"tbuf{i}",
                                   name=f"tbuf{i}")
                    tmps.append(t)
                # A^T chunk tiles [128, 400] x 35 lt x 2 sets.  Local col
                # f = pq + 1 - 396*qc, pq = 66*(py+1) + px + 1 the pitched
                # query col (the pitch keeps dx=+-1 reads from wrapping
                # across image rows).  Pads zeroed once per buffer.
                atT = [[], []]
                atT3 = [[], []]
                for st in range(2):
                    for lt in range(NLT):
                        t = atp.tile([128, 400], _BF16, tag=f"aT{st}_{lt}",
                                     name=f"aT{st}_{lt}")
                        v3 = t[:, 1:397].rearrange("p (a b) -> p a b", a=6)
                        nc.gpsimd.memset(t[:, 0:1], 0.0)
                        nc.gpsimd.memset(t[:, 397:400], 0.0)
                        nc.gpsimd.memset(v3[:, :, 0:1], 0.0)
                        nc.gpsimd.memset(v3[:, :, 65:66], 0.0)
                        atT[st].append(t)
                        atT3[st].append(v3)

                def dy_ops(a):
                    """Emit dy-diagonal-sum ops for A tile a (rows qa =
                    [128a, 128a+128), qa = q + 64).  A[qa, j] =
                    att[qa-128, j] + att[qa-64, j+66] + att[qa, j+132],
                    att rows = core's 1024 queries, att flat width 4488."""
                    A = ats[a % 4]
                    tt = tmps[a % 2]
                    alo = atts[(a - 1) % 3] if a >= 1 else None
                    ahi = atts[a % 3] if a < NM else None
                    flo = alo[:, :, :].rearrange("p a b -> p (a b)") if alo is not None else None
                    fhi = ahi[:, :, :].rearrange("p a b -> p (a b)") if ahi is not None else None
                    eng = [nc.vector, nc.gpsimd]
                    if a == 0:
                        # A[0:64] = att0[0:64, j+132]
                        nc.vector.tensor_copy(A[0:64, DY0:DY1],
                                              fhi[0:64, 132 + DY0:132 + DY1])
                        # A[64:128] = att0[64:128, j+132] + att0[0:64, j+66]
                        nc.vector.tensor_copy(tt[0:64, DY0:DY1],
                                              fhi[64:128, 132 + DY0:132 + DY1])
                        nc.vector.tensor_add(A[64:128, DY0:DY1],
                                             tt[0:64, DY0:DY1],
                                             fhi[0:64, 66 + DY0:66 + DY1])
                    elif a == NM:
                        # A[0:64] = att7[0:64, j] + att7[64:128, j+66]
                        nc.vector.tensor_copy(tt[0:64, DY0:DY1],
                                              flo[64:128, 66 + DY0:66 + DY1])
                        nc.vector.tensor_add(A[0:64, DY0:DY1],
                                             flo[0:64, DY0:DY1],
                                             tt[0:64, DY0:DY1])
                        # A[64:128] = att7[64:128, j]
                        nc.vector.tensor_copy(A[64:128, DY0:DY1],
                                              flo[64:128, DY0:DY1])
                    else:
                        # A[0:64] = att_{a-1}[0:64, j] + att_a[0:64, j+132]
                        #         + att_{a-1}[64:128, j+66]
                        nc.vector.tensor_add(tt[64:128, DY0:DY1],
                                             flo[0:64, DY0:DY1],
                                             fhi[0:64, 132 + DY0:132 + DY1])
                        nc.vector.tensor_add(A[0:64, DY0:DY1],
                                             tt[64:128, DY0:DY1],
                                             flo[64:128, 66 + DY0:66 + DY1])
                        # A[64:128] = att_{a-1}[64:128, j] + att_a[64:128, j+132]
                        #           + att_a[0:64, j+66]
                        nc.vector.tensor_add(tt[0:64, DY0:DY1],
                                             flo[64:128, DY0:DY1],
                                             fhi[64:128, 132 + DY0:132 + DY1])
                        nc.vector.tensor_add(A[64:128, DY0:DY1],
                                             tt[0:64, DY0:DY1],
                                             fhi[0:64, 66 + DY0:66 + DY1])

                def transpose_batch(qc):
                    """Transpose A tiles 3qc..3qc+2 into A^T chunk qc's
                    tiles: one long PE burst (stays in the warm p-state),
                    one wide psum tile + single copy per lt."""
                    for lt in range(NLT):
                        pt = psumT.tile([128, 384], _BF16)
                        for ar in range(3):
                            A = ats[(3 * qc + ar) % 4]
                            nc.tensor.transpose(
                                pt[:, 128 * ar:128 * (ar + 1)],
                                A[:, lt * 128:(lt + 1) * 128], idt[:])
                        psrc = pt[:, :].rearrange("p (a b) -> p a b", a=6)
                        dst = atT3[qc % 2][lt][:, 0:6, 1:65]
                        nc.scalar.activation(
                            dst, psrc,
                            mybir.ActivationFunctionType.Copy, scale=1.0)

                def gemm2_chunk(qc):
                    """y^T[c, pq] = sum_dx sum_lt x1dx_lt^T @
                    A^T_chunk[lt][:, (pq + dx + 1) - 396 qc]."""
                    ps2 = psum2.tile([128, QC], _F32)
                    first = True
                    for lt in range(NLT):
                        for di in range(3):
                            dx = di - 1
                            last = (lt == NLT - 1 and di == 2)
                            nc.tensor.matmul(
                                ps2[:],
                                x1tiles[di][lt],
                                atT[qc % 2][lt][:, dx + 1:dx + 1 + QC],
                                start=first, stop=last)
                            first = False
                    yt = ypool.tile([128, QC], _F32, bufs=2)
                    nc.scalar.activation(
                        yt[:], ps2[:],
                        mybir.ActivationFunctionType.Copy, scale=1.0)
                    nc.sync.dma_start(yout[:, qc * QC:(qc + 1) * QC], yt[:])

                for m in range(NM):
                    sr = srows[m % 5]
                    at = atts[m % 3]
                    if m >= 5:
                        nc.scalar.dma_start(sr[:],
                                            espill[m * 128:(m + 1) * 128, :])
                    if m in (0, 1, 2):
                        load_x1(m)
                    if m >= 3:
                        make_scale(m)
                        for n in range(NCH):
                            seg = at[:, 2 + 8 * n:10 + 8 * n, 1:65]
                            nc.vector.tensor_scalar_mul(
                                seg, sr[:, NL * n:NL * (n + 1)],
                                scales[m][:, n:n + 1])
                    if m == 0:
                        dy_ops(0)
                    if m >= 1:
                        dy_ops(m)
                    if m == NM - 1:
                        dy_ops(NM)
                    # a-tiles 3qc..3qc+2 complete -> transpose burst + GEMM2
                    if m in (2, 5):
                        qc = m // 3
                        transpose_batch(qc)
                        gemm2_chunk(qc)
                    elif m == NM - 1:
                        transpose_batch(2)
                        gemm2_chunk(2)
    nc.compile()
    return nc


def _patches(x):
    """x [H,W,C] -> [H,W,9*C] with (ky,kx) row-major, C innermost; zero pad."""
    Hh, Ww, Cc = x.shape
    xp = np.zeros((Hh + 2, Ww + 2, Cc), x.dtype)
    xp[1:-1, 1:-1] = x
    out = np.empty((Hh, Ww, 9, Cc), x.dtype)
    idx = 0
    for i in range(3):
        for j in range(3):
            out[:, :, idx] = xp[i:i + Hh, j:j + Ww]
            idx += 1
    return out.reshape(Hh, Ww, 9 * Cc)


def _boxsum(p):
    """3x3 spatial box-sum (valid neighbors only) of [H,W,D]."""
    Hh, Ww, D = p.shape
    pp = np.zeros((Hh + 2, Ww + 2, D), p.dtype)
    pp[1:-1, 1:-1] = p
    o = np.zeros_like(p)
    for i in range(3):
        for j in range(3):
            o += pp[i:i + Hh, j:j + Ww]
    return o


def _x1_shift(x1b, dx):
    """Host: padded key-grid copy of x1 for GEMM2.  Row j of the output maps
    to spatial (my, mx) with 66*my + mx = j - 67; value x1[my, mx - dx]."""
    out = np.zeros((AWP, C), np.float32)
    j = np.arange(AWP)
    my = (j - 67) // PG
    mx = (j - 67) - PG * my
    sx = mx - dx
    ok = (my >= 0) & (my < H) & (sx >= 0) & (sx < W)
    out[ok] = x1b[my[ok], sx[ok]]
    return out.astype(ml_dtypes.bfloat16).reshape(NLT, 128, C)


def _make_in_maps(x1, x2):
    cnt = np.full((H, W), 9.0, np.float32)
    cnt[0, :] = cnt[-1, :] = 6.0
    cnt[:, 0] = cnt[:, -1] = 6.0
    cnt[0, 0] = cnt[0, -1] = cnt[-1, 0] = cnt[-1, -1] = 4.0
    ident = np.eye(128, dtype=ml_dtypes.bfloat16)
    in_maps = []
    for b in range(B):
        p2 = _patches(x2[b])                       # [H,W,1152]
        flat = p2.reshape(L, KKC)
        nrm = np.sqrt((flat * flat).sum(-1, keepdims=True))
        wn = flat / np.maximum(nrm, EPS)
        wnT = np.ascontiguousarray(wn.T)           # [1152, 4096]
        qbox = _boxsum(p2) * (SCALE * 9.0 / cnt)[..., None]
        x1s = [_x1_shift(x1[b], dx) for dx in (-1, 0, 1)]
        for s in range(SH):
            qT = np.ascontiguousarray(
                qbox[s * RS:(s + 1) * RS].reshape(MQ, KKC).T)
            in_maps.append({"qboxT": qT, "wnT": wnT,
                            "x1m": x1s[0], "x1z": x1s[1], "x1p": x1s[2],
                            "ident": ident})
    return in_maps


def _make_runner(nc):
    """Build the shard_map executable once; reuse across kernel() calls.

    Mirrors concourse.bass2jax.run_bass_via_pjrt but caches the jitted
    callable so repeated calls skip retracing/relowering.
    """
    import jax
    from jax.sharding import Mesh, PartitionSpec
    from jax.experimental.shard_map import shard_map
    from concourse import bass2jax, mybir as _mb
    bass2jax.install_neuronx_cc_hook()

    partition_name = (nc.partition_id_tensor.name
                      if nc.partition_id_tensor else None)
    in_names, out_names, out_avals, zero_outs = [], [], [], []
    for alloc in nc.m.functions[0].allocations:
        if not isinstance(alloc, _mb.MemoryLocationSet):
            continue
        name = alloc.memorylocations[0].name
        if alloc.kind == "ExternalInput":
            if name != partition_name:
                in_names.append(name)
        elif alloc.kind == "ExternalOutput":
            shape = tuple(alloc.tensor_shape)
            dtype = _mb.dt.np(alloc.dtype)
            out_names.append(name)
            out_avals.append(jax.core.ShapedArray(shape, dtype))
            zero_outs.append(np.zeros(shape, dtype))
    n_params = len(in_names)
    n_outs = len(out_avals)
    all_names = list(in_names) + list(out_names)
    if partition_name is not None:
        all_names.append(partition_name)
    donate = tuple(range(n_params, n_params + n_outs))

    def _body(*args):
        operands = list(args)
        if partition_name is not None:
            operands.append(bass2jax.partition_id_tensor())
        outs = bass2jax._bass_exec_p.bind(
            *operands,
            out_avals=tuple(out_avals),
            in_names=tuple(all_names),
            out_names=tuple(out_names),
            lowering_input_output_aliases=(),
            sim_require_finite=True,
            sim_require_nnan=True,
            nc=nc,
        )
        return tuple(outs)

    devices = jax.devices()[:NCORES]
    mesh = Mesh(np.asarray(devices), ("core",))
    in_specs = (PartitionSpec("core"),) * (n_params + n_outs)
    out_specs = (PartitionSpec("core"),) * n_outs
    sharded = jax.jit(
        shard_map(_body, mesh=mesh, in_specs=in_specs, out_specs=out_specs,
                  check_rep=False),
        donate_argnums=donate, keep_unused=True)

    def run(in_maps):
        concat_in = [
            np.concatenate([np.asarray(in_maps[c][n]) for c in range(NCORES)],
                           axis=0)
            for n in in_names[:n_params]]
        concat_zeros = [
            np.zeros((NCORES * z.shape[0], *z.shape[1:]), z.dtype)
            for z in zero_outs]
        out_arrs = sharded(*concat_in, *concat_zeros)
        return [
            {name: np.asarray(out_arrs[i]).reshape(
                NCORES, *out_avals[i].shape)[c]
             for i, name in enumerate(out_names)}
            for c in range(NCORES)]

    return run


def kernel(x1, x2, mask):
    x1 = np.asarray(x1, np.float32)
    x2 = np.asarray(x2, np.float32)
    if "nc" not in _cache:
        _cache["nc"] = _build()
        try:
            _cache["runner"] = _make_runner(_cache["nc"])
        except Exception:
            _cache["runner"] = None
    nc = _cache["nc"]
    in_maps = _make_in_maps(x1, x2)
    if _cache.get("runner") is not None:
        results = _cache["runner"](in_maps)
    else:
        results = run_bass_kernel_spmd(
            nc, in_maps, core_ids=list(range(NCORES))).results
    y = np.empty((B, H, W, C), np.float32)
    for b in range(B):
        # accumulate the 4 cores' overlapped partial outputs; each core
        # emits y^T[c, pq] (66-pitched) for image rows [16s - 1, 16s + 17).
        yp = np.zeros((H + 2, W, C), np.float32)
        for s in range(SH):
            part = np.asarray(results[b * SH + s]["y"], np.float32)
            blk = part.T.reshape(RS + 2, PG, C)[:, 1:65]   # [18, 64, C]
            yp[16 * s:16 * s + RS + 2] += blk
        y[b] = yp[1:H + 1]
    return y
